# revision 5
# baseline (speedup 1.0000x reference)
"""BiLSTM (2-layer, H=512) Trainium2 Bass kernel.

Contract: kernel(**inputs) takes the FULL unsharded inputs from
setup_inputs() and returns the FULL [32, 512, 1024] float32 output.

Strategy (8 NeuronCores):
  - cores 0-3: forward direction, batch slices 0..3 (8 seqs each)
  - cores 4-7: backward direction, batch slices 0..3
  - two SPMD launches (layer 0, layer 1); host reshuffles between them.
  - backward cores run the IDENTICAL program on time-reversed inputs
    (host flips t), so one Bass program per layer serves all 8 cores.

Math layout per core (B_local=8, T=512, H=512, G=4H=2048):
  - recurrent matmul computed transposed: z^T[G, B] = Wh^T @ h^T via 64
    weights-stationary matmuls [K=128] x [M=128] x [N=8]; gate dim lands
    on partitions so all elementwise gate math runs 128-wide.
  - gates are column-reordered (i, f, o, g) so sigmoid is one [128,96] op.
  - input projection xw^T = Wi^T @ x^T + b is computed into an SBUF fp32
    ring buffer, interleaved with the recurrence to fill PE idle gaps.
  - matmul operands bf16 (fp32 accumulate in PSUM); state c fp32;
    h stored bf16 for the next-step matmul operand.
"""

import os
import sys
from contextlib import ExitStack

import numpy as np

sys.path.insert(0, "/opt/trn_rl_repo")

import ml_dtypes  # noqa: E402

import concourse.bass as bass  # noqa: E402
import concourse.tile as tile  # noqa: E402
from concourse import bacc, mybir  # noqa: E402
from concourse import bass_utils  # noqa: E402

BF16 = mybir.dt.bfloat16
F32 = mybir.dt.float32
NP_BF16 = ml_dtypes.bfloat16
AF = mybir.ActivationFunctionType

B_GLOBAL = 32
T_FULL = 512
D0 = 256
H = 512
G = 4 * H          # 2048
BL = 8             # batch per core
N_MC = 16          # gate-dim chunks of 128
N_KC = 4           # hidden-dim chunks of 128
BLK = 32           # proj block: steps of xw produced per block
RING = 128         # xw ring depth (steps)
HRING = 128        # layer-1 h history ring depth (steps)
HCHUNK = 64        # layer-1 h history DMA-out chunk (steps)
WARM_BLOCKS = 3    # proj blocks emitted before the recurrence starts

# gate reorder: reference order (i, f, g, o) -> kernel order (i, f, o, g)
_PERM = np.concatenate([np.arange(0, 1024), np.arange(1536, 2048),
                        np.arange(1024, 1536)])

_PROGRAM_CACHE = {}

# test hooks: per-launch BassKernelResults (trace mode) / wall seconds
LAST_RESULTS = []
LAST_WALL = []
TRACE = bool(int(os.environ.get("BLSTM_TRACE", "0")))


def _emit_layer(tc, aps, dc_n, T, layer):
    nc = tc.nc
    xT, wh, wi, bT, hout = aps
    ring_depth = min(RING, T)
    hring = min(HRING, T)
    blk = min(BLK, T)
    n_blk = (T + blk - 1) // blk

    ctx = ExitStack()
    const = ctx.enter_context(tc.tile_pool(name="const", bufs=1))
    xin = ctx.enter_context(tc.tile_pool(name="xin", bufs=2 * dc_n))
    pps = ctx.enter_context(tc.tile_pool(name="pps", bufs=2, space="PSUM"))
    rpsA = ctx.enter_context(tc.tile_pool(name="rpsA", bufs=2, space="PSUM"))
    rpsB = ctx.enter_context(tc.tile_pool(name="rpsB", bufs=2, space="PSUM"))
    ztmp = ctx.enter_context(tc.tile_pool(name="ztmp", bufs=3))
    hst = ctx.enter_context(tc.tile_pool(name="hst", bufs=3))

    with ctx:
        # ---- persistent SBUF tensors ----
        wi_sb = []
        for dc in range(dc_n):
            wt = const.tile([128, G], BF16, tag=f"wi{dc}", name=f"wi{dc}")
            nc.sync.dma_start(wt[:], wi[dc])
            wi_sb.append(wt)
        bT_sb = const.tile([128, N_MC], F32, tag="bT", name="bT_sb")
        nc.sync.dma_start(bT_sb[:], bT[:])
        h0 = const.tile([128, 32], BF16, tag="h0", name="h0_sb")
        nc.vector.memset(h0[:], 0.0)
        cT = const.tile([128, 32], F32, tag="cT", name="cT_sb")
        nc.vector.memset(cT[:], 0.0)
        ring = const.tile([128, ring_depth * 128], F32, tag="ring", name="ring_sb")
        if layer == 0:
            hist = const.tile([128, T * 32], BF16, tag="hist", name="hist_sb")
        else:
            hist = const.tile([128, hring * 32], F32, tag="hist", name="hist_sb")
        wh_sb = []
        for kc in range(N_KC):
            wt = const.tile([128, G], BF16, tag=f"wh{kc}", name=f"wh{kc}")
            nc.sync.dma_start(wt[:], wh[kc])
            wh_sb.append(wt)

        ringv = ring.rearrange("p (s c) -> p s c", c=128)

        # ---- projection work generator (one yield per mc-group) ----
        def proj_gen():
            for j in range(n_blk):
                xts = []
                for dc in range(dc_n):
                    xt = xin.tile([128, blk * 8], BF16, tag="xt",
                                  name=f"xt_{j}_{dc}")
                    nc.sync.dma_start(
                        xt[:], xT[dc, :, j * blk * 8:(j + 1) * blk * 8])
                    xts.append(xt)
                s0 = (j * blk) % ring_depth
                for mc in range(N_MC):
                    ps = pps.tile([128, blk * 8], F32, tag="pps",
                                  name=f"pps_{j}_{mc}")
                    for dc in range(dc_n):
                        nc.tensor.matmul(
                            ps[:], wi_sb[dc][:, mc * 128:(mc + 1) * 128],
                            xts[dc][:],
                            start=(dc == 0), stop=(dc == dc_n - 1))
                    psv = ps.rearrange("p (t b) -> p t b", b=8)
                    outv = ringv[:, s0:s0 + blk, mc * 8:(mc + 1) * 8]
                    nc.vector.tensor_scalar_add(outv, psv, bT_sb[:, mc:mc + 1])
                    yield

        gen = proj_gen()
        total_groups = n_blk * N_MC
        warm = min(WARM_BLOCKS * N_MC, total_groups)
        for _ in range(warm):
            next(gen, None)

        prev_state = None  # layer-1 bf16 state tile of previous step

        def rhs(kc, t):
            if t == 0:
                return h0[:, kc * 8:(kc + 1) * 8]
            if layer == 0:
                o = (t - 1) * 32 + kc * 8
                return hist[:, o:o + 8]
            return prev_state[:, kc * 8:(kc + 1) * 8]

        for t in range(T):
            st = t % ring_depth
            # gate g (mc 12..15) first so tanh(g) overlaps the i/f/o matmuls
            psB = rpsB.tile([128, 32], F32, tag="psB", name=f"psB_{t}")
            for i, mc in enumerate(range(12, 16)):
                for kc in range(N_KC):
                    nc.tensor.matmul(
                        psB[:, i * 8:(i + 1) * 8],
                        wh_sb[kc][:, mc * 128:(mc + 1) * 128],
                        rhs(kc, t),
                        start=(kc == 0), stop=(kc == N_KC - 1))
            psA = rpsA.tile([128, 96], F32, tag="psA", name=f"psA_{t}")
            for mc in range(12):
                for kc in range(N_KC):
                    nc.tensor.matmul(
                        psA[:, mc * 8:(mc + 1) * 8],
                        wh_sb[kc][:, mc * 128:(mc + 1) * 128],
                        rhs(kc, t),
                        start=(kc == 0), stop=(kc == N_KC - 1))

            zg = ztmp.tile([128, 32], F32, tag="zg", name=f"zg_{t}")
            nc.vector.tensor_add(zg[:], psB[:],
                                 ring[:, st * 128 + 96:st * 128 + 128])
            zgt = ztmp.tile([128, 32], F32, tag="zgt", name=f"zgt_{t}")
            nc.scalar.activation(zgt[:], zg[:], AF.Tanh)

            zi = ztmp.tile([128, 96], F32, tag="zi", name=f"zi_{t}")
            nc.vector.tensor_add(zi[:], psA[:], ring[:, st * 128:st * 128 + 96])
            za = ztmp.tile([128, 96], F32, tag="za", name=f"za_{t}")
            nc.scalar.activation(za[:], zi[:], AF.Sigmoid)

            ig = ztmp.tile([128, 32], F32, tag="ig", name=f"ig_{t}")
            nc.vector.tensor_mul(ig[:], za[:, 0:32], zgt[:])
            fc = ztmp.tile([128, 32], F32, tag="fc", name=f"fc_{t}")
            nc.vector.tensor_mul(fc[:], za[:, 32:64], cT[:])
            nc.vector.tensor_add(cT[:], fc[:], ig[:])
            tct = ztmp.tile([128, 32], F32, tag="tct", name=f"tct_{t}")
            nc.scalar.activation(tct[:], cT[:], AF.Tanh)

            if layer == 0:
                hs = hist[:, t * 32:(t + 1) * 32]
                nc.vector.tensor_mul(hs, za[:, 64:96], tct[:])
            else:
                hs = hist[:, (t % hring) * 32:((t % hring) + 1) * 32]
                nc.vector.tensor_mul(hs, za[:, 64:96], tct[:])
                stt = hst.tile([128, 32], BF16, tag="hstate", name=f"hstt_{t}")
                nc.vector.tensor_copy(stt[:], hs)
                prev_state = stt
                if (t + 1) % HCHUNK == 0:
                    t0 = t + 1 - HCHUNK
                    c0 = (t0 % hring) * 32
                    nc.sync.dma_start(hout[:, t0 * 32:(t + 1) * 32],
                                      hist[:, c0:c0 + HCHUNK * 32])

            # steady-state projection: one mc-group every other step
            if t % 2 == 0:
                next(gen, None)

        # drain any remaining projection work (shouldn't happen for T=512)
        for _ in gen:
            pass

        if layer == 0:
            nc.sync.dma_start(hout[:], hist[:])
        elif T % HCHUNK != 0:
            t0 = T - (T % HCHUNK)
            c0 = (t0 % hring) * 32
            nc.sync.dma_start(hout[:, t0 * 32:T * 32],
                              hist[:, c0:c0 + (T - t0) * 32])


def build_layer_program(layer, T=T_FULL):
    dc_n = 2 if layer == 0 else 8
    nc = bacc.Bacc("TRN2", target_bir_lowering=False, debug=False,
                   num_devices=8)
    xT = nc.dram_tensor("xT", [dc_n, 128, T * 8], BF16,
                        kind="ExternalInput").ap()
    wh = nc.dram_tensor("wh", [N_KC, 128, G], BF16, kind="ExternalInput").ap()
    wi = nc.dram_tensor("wi", [dc_n, 128, G], BF16, kind="ExternalInput").ap()
    bT = nc.dram_tensor("bT", [128, N_MC], F32, kind="ExternalInput").ap()
    out_dt = BF16 if layer == 0 else F32
    hout = nc.dram_tensor("hout", [128, T * 32], out_dt,
                          kind="ExternalOutput").ap()
    with tile.TileContext(nc) as tc:
        _emit_layer(tc, (xT, wh, wi, bT, hout), dc_n, T, layer)
    nc.compile()
    return nc


def _get_program(layer, T=T_FULL):
    key = (layer, T)
    if key not in _PROGRAM_CACHE:
        _PROGRAM_CACHE[key] = build_layer_program(layer, T)
    return _PROGRAM_CACHE[key]


def _prep_weights(Wi, Wh, b, dc_n):
    wi = np.ascontiguousarray(Wi[:, _PERM]).astype(NP_BF16).reshape(dc_n, 128, G)
    wh = np.ascontiguousarray(Wh[:, _PERM]).astype(NP_BF16).reshape(N_KC, 128, G)
    bT = np.ascontiguousarray(
        b[_PERM].reshape(N_MC, 128).T).astype(np.float32)
    return wi, wh, bT


def _run(nc, in_maps):
    import time
    t0 = time.time()
    res = bass_utils.run_bass_kernel_spmd(
        nc, in_maps, core_ids=list(range(8)), trace=TRACE)
    LAST_WALL.append(time.time() - t0)
    if TRACE:
        LAST_RESULTS.append(res)
    return res.results


def kernel(x, Wi_f0, Wh_f0, b_f0, Wi_b0, Wh_b0, b_b0,
           Wi_f1, Wh_f1, b_f1, Wi_b1, Wh_b1, b_b1):
    T = x.shape[1]
    x = np.asarray(x, dtype=np.float32)

    # ---------------- layer 0 ----------------
    params0 = [_prep_weights(Wi_f0, Wh_f0, b_f0, 2),
               _prep_weights(Wi_b0, Wh_b0, b_b0, 2)]
    nc0 = _get_program(0, T)
    in_maps = []
    for c in range(8):
        d, s = c // 4, c % 4
        xs = x[s * BL:(s + 1) * BL]            # [8, T, 256]
        xt = xs.transpose(2, 1, 0)             # [256, T, 8] = (d, t, b)
        if d == 1:
            xt = xt[:, ::-1, :]
        xt = np.ascontiguousarray(xt).astype(NP_BF16).reshape(2, 128, T * 8)
        wi, wh, bT = params0[d]
        in_maps.append({"xT": xt, "wh": wh, "wi": wi, "bT": bT})
    res0 = _run(nc0, in_maps)

    # assemble layer-1 inputs: hidden0^T = [h_fwd ; h_bwd] along feature dim
    def to_kptb(a, flip):                      # [128, T*32] -> [4, 128, T, 8]
        a = a.reshape(128, T, 4, 8)
        if flip:
            a = a[:, ::-1]
        return a.transpose(2, 0, 1, 3)
    hidden0T = []
    for s in range(4):
        hf = to_kptb(res0[s]["hout"], False)
        hb = to_kptb(res0[4 + s]["hout"], True)
        hidden0T.append(np.concatenate([hf, hb], axis=0))   # [8, 128, T, 8]

    # ---------------- layer 1 ----------------
    params1 = [_prep_weights(Wi_f1, Wh_f1, b_f1, 8),
               _prep_weights(Wi_b1, Wh_b1, b_b1, 8)]
    nc1 = _get_program(1, T)
    in_maps = []
    for c in range(8):
        d, s = c // 4, c % 4
        ht = hidden0T[s]
        if d == 1:
            ht = ht[:, :, ::-1, :]
        xt = np.ascontiguousarray(ht).reshape(8, 128, T * 8)
        wi, wh, bT = params1[d]
        in_maps.append({"xT": xt, "wh": wh, "wi": wi, "bT": bT})
    res1 = _run(nc1, in_maps)

    # ---------------- final assembly ----------------
    out = np.empty((B_GLOBAL, T, 2 * H), np.float32)
    for c in range(8):
        d, s = c // 4, c % 4
        a = res1[c]["hout"].reshape(128, T, 4, 8)
        if d == 1:
            a = a[:, ::-1]
        blk = a.transpose(3, 1, 2, 0).reshape(BL, T, H)
        out[s * BL:(s + 1) * BL, :, d * H:(d + 1) * H] = blk
    return out
